# revision 10
# baseline (speedup 1.0000x reference)
"""DynamicFilter Trainium2 kernel.

Computation (per sample b):
    h  = tanh(query @ W1.T + b1)                      [B, 256]
    cw = (h @ W2.T + b2).reshape(B, C=32, K=31)       per-sample conv weights
    x[b,t,c] = sum_k cw[b,c,k] * pad(prev_attn)[b, t+k]
    out[b,t,o] = sum_c Wfc[o,c] x[b,t,c] + bfc[o]

Key algebraic fusion: fold the fc into the conv,
    Weff[b,o,k] = sum_c Wfc[o,c] cw[b,c,k]            [B, 128, 31]
    out[b,t,o]  = sum_k Weff[b,o,k] pad(prev_attn)[b, t+k] + bfc[o]
so the T-sized work is ONE matmul per (sample, t-chunk):
    psum[128 o, 512 t] = WeffT_b[31 k, 128 o].T @ windows[31 k, 512 t]
with the windows operand streamed straight out of a pre-shifted SBUF
replica tile pa31[k, t] = pad(prev_attn)[b, t + k] (31 shifted copies of
the padded row, so every rhs is a plain rectangular slice).

Sharding: data-parallel over batch. 64 samples / 8 cores = 8 per core.
Weights replicated. Output written [b, o, t] (2 KB contiguous DMA runs),
host returns a transposed view [B, T, O].
"""

import sys

import numpy as np

if "/opt/trn_rl_repo" not in sys.path:
    sys.path.insert(0, "/opt/trn_rl_repo")

from contextlib import ExitStack

import concourse.bass as bass
import concourse.mybir as mybir
import concourse.tile as tile
from concourse import bacc
from concourse.ap import AP
from concourse.bass_utils import run_bass_kernel_spmd

# Problem shapes (hardcoded per contract).
B, T = 64, 4096
D, H = 1024, 256
C, K, O = 32, 31, 128
PAD = (K - 1) // 2  # 15
NCORES = 8
BPC = B // NCORES  # 8 samples per core
PTA = T + 2 * PAD + 2  # padded row length, 4128 (alignment slack)
TCH = 512  # t-chunk (matmul moving free dim)
NT = T // TCH  # 8 chunks per sample

F32 = mybir.dt.float32
F32R = mybir.dt.float32r
AF = mybir.ActivationFunctionType

_CACHED = {}


def _build_nc(use_f32r=True):
    nc = bacc.Bacc(
        "TRN2", target_bir_lowering=False, debug=False, num_devices=NCORES
    )

    qT_h = nc.dram_tensor("qT", [D, BPC], F32, kind="ExternalInput")
    pa_h = nc.dram_tensor("paPad", [BPC, PTA], F32, kind="ExternalInput")
    w1t_h = nc.dram_tensor("w1t", [D, H], F32, kind="ExternalInput")
    b1_h = nc.dram_tensor("b1", [H], F32, kind="ExternalInput")
    w2t_h = nc.dram_tensor("w2t", [H, C * K], F32, kind="ExternalInput")
    b2_h = nc.dram_tensor("b2", [C * K], F32, kind="ExternalInput")
    wfct_h = nc.dram_tensor("wfct", [C, O], F32, kind="ExternalInput")
    bfc_h = nc.dram_tensor("bfc", [O], F32, kind="ExternalInput")
    out_h = nc.dram_tensor("out", [BPC, O, T], F32, kind="ExternalOutput")
    # internal DRAM bounce for the f32r-rounded padded rows
    mmdt = F32R if use_f32r else F32
    par_h = nc.dram_tensor("paPadR", [BPC, PTA], mmdt)

    with tile.TileContext(nc) as tc:
        _emit(tc, qT_h, pa_h, w1t_h, b1_h, w2t_h, b2_h, wfct_h, bfc_h, out_h,
              par_h, use_f32r)

    nc.compile()
    return nc


def _emit(tc, qT_h, pa_h, w1t_h, b1_h, w2t_h, b2_h, wfct_h, bfc_h, out_h,
          par_h, use_f32r):
    nc = tc.nc
    mmdt = F32R if use_f32r else F32
    with ExitStack() as ctx:
        singles = ctx.enter_context(tc.tile_pool(name="singles", bufs=1))
        cw_pool = ctx.enter_context(tc.tile_pool(name="cw", bufs=BPC))
        weff_pool = ctx.enter_context(tc.tile_pool(name="weff", bufs=BPC))
        pa31_pool = ctx.enter_context(tc.tile_pool(name="pa31", bufs=3))
        out_pool = ctx.enter_context(tc.tile_pool(name="outsb", bufs=6))
        psum_pre = ctx.enter_context(
            tc.tile_pool(name="psum_pre", bufs=1, space="PSUM")
        )
        psum_weff = ctx.enter_context(
            tc.tile_pool(name="psum_weff", bufs=2, space="PSUM")
        )
        psum_main = ctx.enter_context(
            tc.tile_pool(name="psum_main", bufs=4, space="PSUM")
        )

        # ---- constant / weight staging -------------------------------
        # w1t_sb[p, (dc, j)] = W1T[128*dc + p, j]   (8 d-chunks of 128)
        w1t_sb = singles.tile([128, 8 * H], F32)
        nc.sync.dma_start(w1t_sb[:], w1t_h.ap().rearrange("(c p) j -> p c j", p=128))
        # w2t_sb[p, (hc, j)] = W2T[128*hc + p, j]   (2 h-chunks of 128)
        w2t_sb = singles.tile([128, 2 * C * K], F32)
        nc.sync.dma_start(w2t_sb[:], w2t_h.ap().rearrange("(c p) j -> p c j", p=128))
        # qt_sb[p, (dc, b)] = qT[128*dc + p, b]
        qt_sb = singles.tile([128, 8 * BPC], F32)
        nc.sync.dma_start(qt_sb[:], qT_h.ap().rearrange("(c p) b -> p c b", p=128))
        wfct_sb = singles.tile([C, O], F32)
        nc.sync.dma_start(wfct_sb[:], wfct_h.ap())
        b1_sb = singles.tile([128, 2], F32)
        nc.sync.dma_start(b1_sb[:], b1_h.ap().rearrange("(c p) -> p c", p=128))
        b2_sb = singles.tile([1, C * K], F32)
        nc.sync.dma_start(b2_sb[:], b2_h.ap().unsqueeze(0))
        bfc_sb = singles.tile([O, 1], F32)
        nc.sync.dma_start(bfc_sb[:], bfc_h.ap().unsqueeze(1))
        ones_sb = singles.tile([1, BPC], F32)
        nc.gpsimd.memset(ones_sb[:], 1.0)

        # ---- padded rows: load, round to matmul dtype, bounce to DRAM ----
        # (the walrus verifier requires fp32r matmul operands to come from
        # an explicit rounding op, so the cast happens on DVE here)
        pa_sb = singles.tile([BPC, PTA], F32)
        nc.sync.dma_start(pa_sb[:], pa_h.ap())
        pa_r = singles.tile([BPC, PTA], mmdt)
        nc.vector.tensor_copy(pa_r[:], pa_sb[:])
        nc.sync.dma_start(par_h.ap(), pa_r[:])

        # ---- hypernet mm1: hT[j, b] = tanh(sum_d W1T[d, j] qT[d, b] + b1) --
        ht_sb = singles.tile([128, 2 * BPC], F32)
        for jc in range(2):
            ph = psum_pre.tile([128, BPC], F32, tag="ph")
            for dc in range(8):
                nc.tensor.matmul(
                    ph[:],
                    lhsT=w1t_sb[:, H * dc + 128 * jc : H * dc + 128 * jc + 128],
                    rhs=qt_sb[:, BPC * dc : BPC * dc + BPC],
                    start=(dc == 0),
                    stop=(dc == 7),
                )
            nc.scalar.activation(
                ht_sb[:, BPC * jc : BPC * jc + BPC], ph[:], AF.Tanh,
                bias=b1_sb[:, jc : jc + 1],
            )

        # ---- hypernet mm2: cwB[b, (c k)] = sum_h W2T[h, ck] hT[h, b] + b2 --
        cwB_sb = singles.tile([BPC, C * K], F32)
        HALF = C * K // 2  # 496
        for nh in range(2):
            pc = psum_pre.tile([BPC, HALF], F32, tag="pc")
            for hc in range(2):
                nc.tensor.matmul(
                    pc[:],
                    lhsT=ht_sb[:, BPC * hc : BPC * hc + BPC],
                    rhs=w2t_sb[:, C * K * hc + HALF * nh : C * K * hc + HALF * nh + HALF],
                    start=(hc == 0),
                    stop=False,
                )
            # bias row: K=1 accumulating matmul with a ones stationary
            nc.tensor.matmul(
                pc[:],
                lhsT=ones_sb[:],
                rhs=b2_sb[:, HALF * nh : HALF * nh + HALF],
                start=False,
                stop=True,
            )
            nc.vector.tensor_copy(cwB_sb[:, HALF * nh : HALF * nh + HALF], pc[:])

        # ---- per-sample cw gather: cw_b[c, k] <- cwB[b, 31c + k] ------
        cw_tiles = []
        for b in range(BPC):
            cwt = cw_pool.tile([C, K], F32, tag="cwt")
            nc.sync.dma_start(
                cwt[:], cwB_sb[b : b + 1, :].rearrange("p (c k) -> p c k", c=C)
            )
            cw_tiles.append(cwt)

        # ---- Weff: WeffT_b[k, o] = sum_c cw_b[c, k] WfcT[c, o] --------
        weff_tiles = []
        for b in range(BPC):
            pw = psum_weff.tile([K, O], F32, tag="pweff")
            nc.tensor.matmul(
                pw[:], lhsT=cw_tiles[b][:], rhs=wfct_sb[:], start=True, stop=True
            )
            wt = weff_pool.tile([K, O], mmdt, tag="weff")
            nc.vector.tensor_copy(wt[:], pw[:])
            weff_tiles.append(wt)

        # ---- main loop: out[b, :, t0:t0+512] = WeffT_b.T @ windows ----
        idx = 0
        out_ap = out_h.ap()
        for b in range(BPC):
            # pa31[k, t] = paPadR[b, k + t]  (31 shifted replicas, 16 KB each)
            pa31 = pa31_pool.tile([K, T], mmdt, tag="pa31")
            nc.sync.dma_start(pa31[:], AP(par_h, b * PTA, [[1, K], [1, T]]))
            for tcn in range(NT):
                pm = psum_main.tile([O, TCH], F32, tag="pmm")
                nc.tensor.matmul(
                    pm[:],
                    lhsT=weff_tiles[b][:],
                    rhs=pa31[:, TCH * tcn : TCH * tcn + TCH],
                    start=True,
                    stop=True,
                )
                osb = out_pool.tile([O, TCH], F32, tag="osb")
                # psum -> sbuf with +bfc, alternating ACT / DVE
                if idx % 2 == 0:
                    nc.scalar.activation(osb[:], pm[:], AF.Identity,
                                         bias=bfc_sb[:, 0:1])
                else:
                    nc.vector.tensor_scalar_add(osb[:], pm[:], bfc_sb[:, 0:1])
                nc.sync.dma_start(
                    out_ap[b, :, TCH * tcn : TCH * tcn + TCH], osb[:]
                )
                idx += 1


def get_nc(use_f32r=True):
    key = ("nc", use_f32r)
    if key not in _CACHED:
        _CACHED[key] = _build_nc(use_f32r)
    return _CACHED[key]


def make_in_maps(query, prev_attn, W1, b1, W2, b2, Wfc, bfc):
    """Shard + lay out host inputs for the 8 cores."""
    f = np.float32
    w1t = np.ascontiguousarray(np.asarray(W1, f).T)  # [D, H]
    w2t = np.ascontiguousarray(np.asarray(W2, f).T)  # [H, C*K]
    wfct = np.ascontiguousarray(np.asarray(Wfc, f).T)  # [C, O]
    b1 = np.ascontiguousarray(np.asarray(b1, f))
    b2 = np.ascontiguousarray(np.asarray(b2, f))
    bfc = np.ascontiguousarray(np.asarray(bfc, f))
    query = np.asarray(query, f)
    prev_attn = np.asarray(prev_attn, f)

    in_maps = []
    for i in range(NCORES):
        sl = slice(i * BPC, (i + 1) * BPC)
        qT = np.ascontiguousarray(query[sl].T)  # [D, BPC]
        pa = np.zeros((BPC, PTA), f)
        pa[:, PAD : PAD + T] = prev_attn[sl]
        in_maps.append(
            {
                "qT": qT,
                "paPad": pa,
                "w1t": w1t,
                "b1": b1,
                "w2t": w2t,
                "b2": b2,
                "wfct": wfct,
                "bfc": bfc,
            }
        )
    return in_maps


def assemble_output(results):
    """[8 cores] x [BPC, O, T] -> [B, T, O] view."""
    full = np.concatenate([r["out"] for r in results], axis=0)  # [B, O, T]
    return full.transpose(0, 2, 1)


def kernel(query, prev_attn, W1, b1, W2, b2, Wfc, bfc):
    nc = get_nc(use_f32r=True)
    in_maps = make_in_maps(query, prev_attn, W1, b1, W2, b2, Wfc, bfc)
    res = run_bass_kernel_spmd(nc, in_maps, list(range(NCORES)))
    return assemble_output(res.results)


# revision 11
# speedup vs baseline: 1.0459x; 1.0459x over previous
"""DynamicFilter Trainium2 kernel.

Computation (per sample b):
    h  = tanh(query @ W1.T + b1)                      [B, 256]
    cw = (h @ W2.T + b2).reshape(B, C=32, K=31)       per-sample conv weights
    x[b,t,c] = sum_k cw[b,c,k] * pad(prev_attn)[b, t+k]
    out[b,t,o] = sum_c Wfc[o,c] x[b,t,c] + bfc[o]

Key algebraic fusion: fold the fc into the conv,
    Weff[b,o,k] = sum_c Wfc[o,c] cw[b,c,k]            [B, 128, 31]
    out[b,t,o]  = sum_k Weff[b,o,k] pad(prev_attn)[b, t+k] + bfc[o]
so the T-sized work is ONE fp32r matmul per (sample, 512-wide t-chunk):
    psum[128 o, 512 t] = WeffT_b[31 k, 128 o].T @ windows[31 k, 512 t]
with the windows operand streamed straight out of pre-shifted SBUF
replica rows pa[32*i + k, t] = pad(prev_attn)[b, t + k].

The replicas are built 3 samples per tile (partition bases 0/32/64 -- the
matmul operand base-partition constraint) with a single SBUF->SBUF DMA
whose 96-partition destination spreads descriptors across the 16 SDMA
engines (a 31-partition destination serializes onto one engine).

Sharding: data-parallel over batch. 64 samples / 8 cores = 8 per core.
Weights replicated. Output written [b, o, t] (2 KB contiguous DMA runs),
host returns a transposed view [B, T, O].
"""

import sys

import numpy as np

if "/opt/trn_rl_repo" not in sys.path:
    sys.path.insert(0, "/opt/trn_rl_repo")

from contextlib import ExitStack

import concourse.bass as bass
import concourse.mybir as mybir
import concourse.tile as tile
from concourse import bacc
from concourse.ap import AP
from concourse.bass_utils import run_bass_kernel_spmd

# Problem shapes (hardcoded per contract).
B, T = 64, 4096
D, H = 1024, 256
C, K, O = 32, 31, 128
PAD = (K - 1) // 2  # 15
NCORES = 8
BPC = B // NCORES  # 8 samples per core
PTA = T + 2 * PAD + 2  # padded row length, 4128 (k+t reads stay in-row)
TCH = 512  # t-chunk (matmul moving free dim)
NT = T // TCH  # 8 chunks per sample
GROUPS = [(0, 3), (3, 3), (6, 2)]  # (first sample, count) per replica tile

F32 = mybir.dt.float32
F32R = mybir.dt.float32r
AF = mybir.ActivationFunctionType

_CACHED = {}


def _build_nc(use_f32r=True):
    nc = bacc.Bacc(
        "TRN2", target_bir_lowering=False, debug=False, num_devices=NCORES
    )

    qT_h = nc.dram_tensor("qT", [D, BPC], F32, kind="ExternalInput")
    pa_h = nc.dram_tensor("paPad", [BPC, PTA], F32, kind="ExternalInput")
    w1t_h = nc.dram_tensor("w1t", [D, H], F32, kind="ExternalInput")
    b1_h = nc.dram_tensor("b1", [H], F32, kind="ExternalInput")
    w2t_h = nc.dram_tensor("w2t", [H, C * K], F32, kind="ExternalInput")
    b2_h = nc.dram_tensor("b2", [C * K], F32, kind="ExternalInput")
    wfct_h = nc.dram_tensor("wfct", [C, O], F32, kind="ExternalInput")
    bfc_h = nc.dram_tensor("bfc", [O], F32, kind="ExternalInput")
    out_h = nc.dram_tensor("out", [BPC, O, T], F32, kind="ExternalOutput")

    with tile.TileContext(nc) as tc:
        _emit(tc, qT_h, pa_h, w1t_h, b1_h, w2t_h, b2_h, wfct_h, bfc_h, out_h,
              use_f32r)

    nc.compile()
    return nc


def _emit(tc, qT_h, pa_h, w1t_h, b1_h, w2t_h, b2_h, wfct_h, bfc_h, out_h,
          use_f32r):
    nc = tc.nc
    mmdt = F32R if use_f32r else F32
    with ExitStack() as ctx:
        singles = ctx.enter_context(tc.tile_pool(name="singles", bufs=1))
        cw_pool = ctx.enter_context(tc.tile_pool(name="cw", bufs=BPC))
        weff_pool = ctx.enter_context(tc.tile_pool(name="weff", bufs=3))
        pa_pool = ctx.enter_context(tc.tile_pool(name="pa", bufs=2))
        out_pool = ctx.enter_context(tc.tile_pool(name="outsb", bufs=6))
        psum_pre = ctx.enter_context(
            tc.tile_pool(name="psum_pre", bufs=1, space="PSUM")
        )
        psum_weff = ctx.enter_context(
            tc.tile_pool(name="psum_weff", bufs=2, space="PSUM")
        )
        psum_main = ctx.enter_context(
            tc.tile_pool(name="psum_main", bufs=4, space="PSUM")
        )

        # ---- constant / weight staging -------------------------------
        # w1t_sb[p, (dc, j)] = W1T[128*dc + p, j]   (8 d-chunks of 128)
        w1t_sb = singles.tile([128, 8 * H], F32)
        nc.sync.dma_start(w1t_sb[:], w1t_h.ap().rearrange("(c p) j -> p c j", p=128))
        # w2t_sb[p, (hc, j)] = W2T[128*hc + p, j]   (2 h-chunks of 128)
        w2t_sb = singles.tile([128, 2 * C * K], F32)
        nc.sync.dma_start(w2t_sb[:], w2t_h.ap().rearrange("(c p) j -> p c j", p=128))
        # qt_sb[p, (dc, b)] = qT[128*dc + p, b]
        qt_sb = singles.tile([128, 8 * BPC], F32)
        nc.sync.dma_start(qt_sb[:], qT_h.ap().rearrange("(c p) b -> p c b", p=128))
        wfct_sb = singles.tile([C, O], F32)
        nc.sync.dma_start(wfct_sb[:], wfct_h.ap())
        b1_sb = singles.tile([128, 2], F32)
        nc.sync.dma_start(b1_sb[:], b1_h.ap().rearrange("(c p) -> p c", p=128))
        b2_sb = singles.tile([1, C * K], F32)
        nc.sync.dma_start(b2_sb[:], b2_h.ap().unsqueeze(0))
        bfc_sb = singles.tile([O, 1], F32)
        nc.sync.dma_start(bfc_sb[:], bfc_h.ap().unsqueeze(1))
        ones_sb = singles.tile([1, BPC], F32)
        nc.gpsimd.memset(ones_sb[:], 1.0)

        # ---- padded rows: load + round to the matmul dtype -----------
        # (the walrus verifier requires fp32r matmul operands to come
        # from an explicit rounding op, so the cast happens on DVE)
        pa_sb = singles.tile([BPC, PTA], F32)
        nc.sync.dma_start(pa_sb[:], pa_h.ap())
        pa_r = singles.tile([BPC, PTA], mmdt)
        nc.vector.tensor_copy(pa_r[:], pa_sb[:])

        # ---- hypernet mm1: hT[j, b] = tanh(sum_d W1T[d, j] qT[d, b] + b1) --
        ht_sb = singles.tile([128, 2 * BPC], F32)
        for jc in range(2):
            ph = psum_pre.tile([128, BPC], F32, tag="ph")
            for dc in range(8):
                nc.tensor.matmul(
                    ph[:],
                    lhsT=w1t_sb[:, H * dc + 128 * jc : H * dc + 128 * jc + 128],
                    rhs=qt_sb[:, BPC * dc : BPC * dc + BPC],
                    start=(dc == 0),
                    stop=(dc == 7),
                )
            nc.scalar.activation(
                ht_sb[:, BPC * jc : BPC * jc + BPC], ph[:], AF.Tanh,
                bias=b1_sb[:, jc : jc + 1],
            )

        # ---- hypernet mm2: cwB[b, (c k)] = sum_h W2T[h, ck] hT[h, b] + b2 --
        cwB_sb = singles.tile([BPC, C * K], F32)
        HALF = C * K // 2  # 496
        for nh in range(2):
            pc = psum_pre.tile([BPC, HALF], F32, tag="pc")
            for hc in range(2):
                nc.tensor.matmul(
                    pc[:],
                    lhsT=ht_sb[:, BPC * hc : BPC * hc + BPC],
                    rhs=w2t_sb[:, C * K * hc + HALF * nh : C * K * hc + HALF * nh + HALF],
                    start=(hc == 0),
                    stop=False,
                )
            # bias row: K=1 accumulating matmul with a ones stationary
            nc.tensor.matmul(
                pc[:],
                lhsT=ones_sb[:],
                rhs=b2_sb[:, HALF * nh : HALF * nh + HALF],
                start=False,
                stop=True,
            )
            nc.vector.tensor_copy(cwB_sb[:, HALF * nh : HALF * nh + HALF], pc[:])

        # ---- per-sample cw gather: cw_b[c, k] <- cwB[b, 31c + k] ------
        cw_tiles = []
        for b in range(BPC):
            cwt = cw_pool.tile([C, K], F32, tag="cwt")
            nc.sync.dma_start(
                cwt[:], cwB_sb[b : b + 1, :].rearrange("p (c k) -> p c k", c=C)
            )
            cw_tiles.append(cwt)

        # ---- Weff per group: WeffT_b[k, o] = sum_c cw_b[c, k] WfcT[c, o] ---
        # sample i of a group lives at partition base 32*i (matmul operand
        # bases are restricted to {0, 32, 64})
        weff_tiles = []
        for b0, cnt in GROUPS:
            pw = psum_weff.tile([96, O], F32, tag="pweff")
            for i in range(cnt):
                nc.tensor.matmul(
                    pw[32 * i : 32 * i + K, :],
                    lhsT=cw_tiles[b0 + i][:],
                    rhs=wfct_sb[:],
                    start=True,
                    stop=True,
                )
            wg = weff_pool.tile([96, O], mmdt, tag="weff")
            for i in range(cnt):
                nc.vector.tensor_copy(
                    wg[32 * i : 32 * i + K, :], pw[32 * i : 32 * i + K, :]
                )
            weff_tiles.append(wg)

        # ---- main loop ------------------------------------------------
        idx = 0
        out_ap = out_h.ap()
        for gi, (b0, cnt) in enumerate(GROUPS):
            # pa_g[32*i + k, t] = paPad[b0 + i, k + t]: shifted replicas of
            # cnt samples, one 96-partition SBUF->SBUF DMA (descriptors
            # spread across SDMA engines by destination partition)
            pa_g = pa_pool.tile([96, T], mmdt, tag="pa")
            src = AP(pa_r.tensor, b0 * PTA, [[PTA, cnt], [1, 32], [1, T]])
            nc.sync.dma_start(pa_g[0 : 32 * cnt, :], src)
            wg = weff_tiles[gi]
            for i in range(cnt):
                lhsT = wg[32 * i : 32 * i + K, :]
                b = b0 + i
                for tcn in range(NT):
                    pm = psum_main.tile([O, TCH], F32, tag="pmm")
                    nc.tensor.matmul(
                        pm[:],
                        lhsT=lhsT,
                        rhs=pa_g[32 * i : 32 * i + K, TCH * tcn : TCH * tcn + TCH],
                        start=True,
                        stop=True,
                    )
                    osb = out_pool.tile([O, TCH], F32, tag="osb")
                    # psum -> sbuf with +bfc, alternating ACT / DVE
                    if idx % 2 == 0:
                        nc.scalar.activation(osb[:], pm[:], AF.Identity,
                                             bias=bfc_sb[:, 0:1])
                    else:
                        nc.vector.tensor_scalar_add(osb[:], pm[:], bfc_sb[:, 0:1])
                    nc.sync.dma_start(
                        out_ap[b, :, TCH * tcn : TCH * tcn + TCH], osb[:]
                    )
                    idx += 1


def get_nc(use_f32r=True):
    key = ("nc", use_f32r)
    if key not in _CACHED:
        _CACHED[key] = _build_nc(use_f32r)
    return _CACHED[key]


def make_in_maps(query, prev_attn, W1, b1, W2, b2, Wfc, bfc):
    """Shard + lay out host inputs for the 8 cores."""
    f = np.float32
    w1t = np.ascontiguousarray(np.asarray(W1, f).T)  # [D, H]
    w2t = np.ascontiguousarray(np.asarray(W2, f).T)  # [H, C*K]
    wfct = np.ascontiguousarray(np.asarray(Wfc, f).T)  # [C, O]
    b1 = np.ascontiguousarray(np.asarray(b1, f))
    b2 = np.ascontiguousarray(np.asarray(b2, f))
    bfc = np.ascontiguousarray(np.asarray(bfc, f))
    query = np.asarray(query, f)
    prev_attn = np.asarray(prev_attn, f)

    in_maps = []
    for i in range(NCORES):
        sl = slice(i * BPC, (i + 1) * BPC)
        qT = np.ascontiguousarray(query[sl].T)  # [D, BPC]
        pa = np.zeros((BPC, PTA), f)
        pa[:, PAD : PAD + T] = prev_attn[sl]
        in_maps.append(
            {
                "qT": qT,
                "paPad": pa,
                "w1t": w1t,
                "b1": b1,
                "w2t": w2t,
                "b2": b2,
                "wfct": wfct,
                "bfc": bfc,
            }
        )
    return in_maps


def assemble_output(results):
    """[8 cores] x [BPC, O, T] -> [B, T, O] view."""
    full = np.concatenate([r["out"] for r in results], axis=0)  # [B, O, T]
    return full.transpose(0, 2, 1)


def kernel(query, prev_attn, W1, b1, W2, b2, Wfc, bfc):
    nc = get_nc(use_f32r=True)
    in_maps = make_in_maps(query, prev_attn, W1, b1, W2, b2, Wfc, bfc)
    res = run_bass_kernel_spmd(nc, in_maps, list(range(NCORES)))
    return assemble_output(res.results)
